# revision 10
# baseline (speedup 1.0000x reference)
"""Trainium2 Bass kernel for nn_Net_5488968204310 (gnn_message_passing).

Single-head self-attention (D=128) over N=1024 nodes + gated residual update,
batch B=32, data-parallel across 8 NeuronCores (4 samples per core).

Layout strategy per sample (all on one core):
  - "T layout": features d on partitions, nodes n on the free dim. All eight
    128x128 weight matmuls contract over d, so they want T-layout operands.
  - QK^T is computed as logitsT chunks [m_chunk(128) x q(1024)] = kT_chunk.T @ qT.
  - exp() runs on the scalar engine straight out of PSUM (scale=1/sqrt(D) folded
    into the activation), producing bf16 attention weights expw.
  - AV uses expw chunks as the *stationary* operand against a moving operand
    v_aug = [v | ones] of width 129: out[q,129] accumulates both attn_unnorm
    (cols 0..127) and the softmax denominator (col 128) in one pass, in natural
    layout where the denominator is a per-partition scalar -> normalization is
    a free per-partition scale on the scalar engine.
  - Normalized attn is transposed back to T layout for the output/gate matmuls;
    the final gated delta is transposed back to natural layout and added to x
    in fp32.

Host-side preprocessing folds biases: bv is folded into the msg bias via
bv @ Wo, Wo1 is replaced by (Wo1 - I) so that ret - x comes out of the matmul
directly, and the sigmoid is computed as 0.5 + 0.5*tanh(z/2) (tanh shares the
ACT table set with exp, avoiding table thrash). Bias handling is specialized
at build time on the actual values (zero -> plain copy, uniform -> immediate,
general -> ACT bias AP) because walrus rejects DVE tensor_scalar with an AP
scalar here ("too many sync wait commands").
"""

import math

import numpy as np
import ml_dtypes

B, N, D = 32, 1024, 128
NCORES = 8
BPC = B // NCORES  # samples per core
NT = N // 128      # node chunks per sample

_CACHE = {}


def _bias_mode(vec):
    """(kind, value) where kind in {'zero', 'uniform', 'ap'}."""
    v = np.asarray(vec, np.float32)
    if not np.any(v):
        return ("zero", 0.0)
    if np.all(v == v.flat[0]):
        return ("uniform", float(v.flat[0]))
    return ("ap", 0.0)


def _build_nc(modes):
    import concourse.bacc as bacc
    import concourse.tile as tile
    from concourse import mybir
    from concourse import masks
    from contextlib import ExitStack

    f32 = mybir.dt.float32
    bf16 = mybir.dt.bfloat16
    AF = mybir.ActivationFunctionType
    OP = mybir.AluOpType

    nc = bacc.Bacc("TRN2", target_bir_lowering=False, debug=False)

    x_d = nc.dram_tensor("x", [BPC, N, D], f32, kind="ExternalInput")
    out_d = nc.dram_tensor("out", [BPC, N, D], f32, kind="ExternalOutput")
    wnames = ["Wq", "Wk", "Wv", "Wo", "Wo1m", "Wg1", "Wg2", "Wg3"]
    w_d = {n: nc.dram_tensor(n, [D, D], bf16, kind="ExternalInput") for n in wnames}
    b_d = {
        n: nc.dram_tensor(n, [D, 1], f32, kind="ExternalInput")
        for n in modes if modes[n][0] == "ap"
    }

    s = 1.0 / math.sqrt(D)

    with tile.TileContext(nc) as tc, ExitStack() as ctx:
        consts = ctx.enter_context(tc.tile_pool(name="consts", bufs=1))
        sb = ctx.enter_context(tc.tile_pool(name="sb", bufs=2))
        expp = ctx.enter_context(tc.tile_pool(name="expp", bufs=2))
        pw = ctx.enter_context(tc.tile_pool(name="pw", bufs=2, space="PSUM"))
        ptr = ctx.enter_context(tc.tile_pool(name="ptr", bufs=2, space="PSUM"))
        pat = ctx.enter_context(tc.tile_pool(name="pat", bufs=2, space="PSUM"))

        W = {}
        for n in wnames:
            t = consts.tile([D, D], bf16, tag=f"w_{n}")
            nc.sync.dma_start(t, w_d[n][:, :])
            W[n] = t
        BV = {}
        for n in b_d:
            t = consts.tile([D, 1], f32, tag=f"b_{n}")
            nc.sync.dma_start(t, b_d[n][:, :])
            BV[n] = t
        ident = consts.tile([128, 128], bf16, tag="ident")
        masks.make_identity(nc, ident)
        for n, (kind, val) in modes.items():
            if kind == "uniform":
                t = consts.tile([D, 1], f32, tag=f"b_{n}")
                nc.vector.memset(t, val)
                BV[n] = t

        def copyback(dst, src, bname, engine_copy):
            """psum->sbuf copy honoring the bias mode for `bname`."""
            kind, val = modes[bname]
            if kind == "zero":
                engine_copy(dst, src)
            else:
                nc.scalar.activation(dst, src, AF.Identity, bias=BV[bname])

        def act_bias(bname):
            kind, val = modes[bname]
            return 0.0 if kind == "zero" else BV[bname]

        for b in range(BPC):
            # ---- load x[b] natural: [n%128, n//128, d] ----
            x_nat = sb.tile([128, NT, D], f32, tag="x_nat")
            nc.sync.dma_start(x_nat, x_d[b].rearrange("(c p) d -> p c d", p=128))
            x_bf = sb.tile([128, NT, D], bf16, tag="x_bf")
            nc.gpsimd.tensor_copy(x_bf, x_nat)

            # ---- xT: transpose each chunk on PE ----
            p_xt = ptr.tile([128, NT, 128], bf16, tag="ptr_bf")
            for c in range(NT):
                nc.tensor.transpose(p_xt[:, c, :], x_bf[:, c, :], ident)
            xT = sb.tile([128, NT, 128], bf16, tag="xT")  # [d, c, n']
            nc.vector.tensor_copy(xT, p_xt)
            xT2 = xT.rearrange("p c n -> p (c n)")  # [d, n]

            # ---- q,k projections (T layout) ----
            p_q = pw.tile([128, N], f32, tag="pw")
            nc.tensor.matmul(p_q[:, 0:512], W["Wq"], xT2[:, 0:512], start=True, stop=True)
            nc.tensor.matmul(p_q[:, 512:1024], W["Wq"], xT2[:, 512:1024], start=True, stop=True)
            qT = sb.tile([128, N], bf16, tag="qT")
            copyback(qT, p_q, "bq", nc.vector.tensor_copy)

            p_k = pw.tile([128, N], f32, tag="pw")
            nc.tensor.matmul(p_k[:, 0:512], W["Wk"], xT2[:, 0:512], start=True, stop=True)
            nc.tensor.matmul(p_k[:, 512:1024], W["Wk"], xT2[:, 512:1024], start=True, stop=True)
            kT = sb.tile([128, N], bf16, tag="kT")
            copyback(kT, p_k, "bk", nc.vector.tensor_copy)

            # ---- v projection, natural layout, with ones column ----
            p_v = pw.tile([128, N], f32, tag="pw")
            for c in range(NT):
                nc.tensor.matmul(p_v[:, c * 128:(c + 1) * 128], xT[:, c, :], W["Wv"], start=True, stop=True)
            v_aug = sb.tile([128, NT, 129], bf16, tag="v_aug")
            nc.vector.tensor_copy(v_aug[:, :, 0:128], p_v.rearrange("p (c n) -> p c n", c=NT))
            nc.gpsimd.memset(v_aug[:, :, 128:129], 1.0)

            # ---- QK^T + exp (softmax numerator), chunked over m ----
            expw = expp.tile([128, NT, N], bf16, tag="expw")  # [m', c_m, q]
            for c in range(NT):
                p_l = pw.tile([128, N], f32, tag="pw")
                kTc = kT[:, c * 128:(c + 1) * 128]
                nc.tensor.matmul(p_l[:, 0:512], kTc, qT[:, 0:512], start=True, stop=True)
                nc.tensor.matmul(p_l[:, 512:1024], kTc, qT[:, 512:1024], start=True, stop=True)
                nc.scalar.activation(expw[:, c, :], p_l, AF.Exp, scale=s)

            # ---- AV + denominator, then normalize + transpose back to T ----
            attn_n = sb.tile([128, NT, 128], bf16, tag="attn_n")  # [q', qc, d]
            for qc in range(NT):
                p_a = pat.tile([128, 129], f32, tag="pat")
                for c in range(NT):
                    nc.tensor.matmul(
                        p_a,
                        expw[:, c, qc * 128:(qc + 1) * 128],
                        v_aug[:, c, :],
                        start=(c == 0),
                        stop=(c == NT - 1),
                    )
                r = sb.tile([128, 1], f32, tag="r")
                nc.vector.reciprocal(r, p_a[:, 128:129])
                nc.scalar.mul(attn_n[:, qc, :], p_a[:, 0:128], r)

            p_at = ptr.tile([128, NT, 128], bf16, tag="ptr_bf")
            for qc in range(NT):
                nc.tensor.transpose(p_at[:, qc, :], attn_n[:, qc, :], ident)
            attnT = sb.tile([128, NT, 128], bf16, tag="attnT")  # [d, qc, q']
            nc.vector.tensor_copy(attnT, p_at)
            attnT2 = attnT.rearrange("p c n -> p (c n)")

            # ---- msg projection ----
            p_m = pw.tile([128, N], f32, tag="pw")
            nc.tensor.matmul(p_m[:, 0:512], W["Wo"], attnT2[:, 0:512], start=True, stop=True)
            nc.tensor.matmul(p_m[:, 512:1024], W["Wo"], attnT2[:, 512:1024], start=True, stop=True)
            msgT = sb.tile([128, N], bf16, tag="msgT")
            copyback(msgT, p_m, "bo_msg", nc.scalar.copy)
            msgu = sb.tile([128, N], f32, tag="msgu")
            copyback(msgu, p_m, "bo_u", nc.vector.tensor_copy)

            # ---- ret - x  (Wo1 - I folded host-side) ----
            p_o1 = pw.tile([128, N], f32, tag="pw")
            nc.tensor.matmul(p_o1[:, 0:512], W["Wo1m"], xT2[:, 0:512], start=True, stop=True)
            nc.tensor.matmul(p_o1[:, 512:1024], W["Wo1m"], xT2[:, 512:1024], start=True, stop=True)
            u = sb.tile([128, N], f32, tag="u")
            nc.vector.tensor_add(u, p_o1, msgu)

            # ---- gate pre-activation: relu(x@Wg1 + msg@Wg2 + bg1 + bg2) ----
            p_g = pw.tile([128, N], f32, tag="pw")
            for h in range(2):
                sl = slice(h * 512, (h + 1) * 512)
                nc.tensor.matmul(p_g[:, sl], W["Wg1"], xT2[:, sl], start=True, stop=False)
                nc.tensor.matmul(p_g[:, sl], W["Wg2"], msgT[:, sl], start=False, stop=True)
            gp = sb.tile([128, N], bf16, tag="gp")
            nc.scalar.activation(gp, p_g, AF.Relu, bias=act_bias("bg12"))

            # ---- gate: sigmoid(z) = 0.5 + 0.5*tanh(z/2) ----
            p_g3 = pw.tile([128, N], f32, tag="pw")
            nc.tensor.matmul(p_g3[:, 0:512], W["Wg3"], gp[:, 0:512], start=True, stop=True)
            nc.tensor.matmul(p_g3[:, 512:1024], W["Wg3"], gp[:, 512:1024], start=True, stop=True)
            tT = sb.tile([128, N], f32, tag="tT")
            nc.scalar.activation(tT, p_g3, AF.Tanh, scale=0.5, bias=act_bias("bg3h"))
            gate = sb.tile([128, N], f32, tag="gate")
            nc.gpsimd.tensor_scalar(gate, tT, 0.5, 0.5, op0=OP.mult, op1=OP.add)

            # ---- delta = gate * (ret - x), back to natural, add x, store ----
            dlt = sb.tile([128, NT, 128], bf16, tag="dlt")
            nc.gpsimd.tensor_tensor(dlt.rearrange("p c n -> p (c n)"), u, gate, op=OP.mult)
            p_d = ptr.tile([128, NT, 128], bf16, tag="ptr_bf")
            for c in range(NT):
                nc.tensor.transpose(p_d[:, c, :], dlt[:, c, :], ident)
            o = sb.tile([128, NT, D], f32, tag="o")
            nc.vector.tensor_add(o, p_d, x_nat)
            nc.sync.dma_start(out_d[b].rearrange("(c p) d -> p c d", p=128), o)

    nc.compile()
    return nc


def _prep_host(inputs):
    """Host-side: fold weights/biases; returns (weights bf16, bias vectors f32)."""
    f32 = np.float32
    bf16 = ml_dtypes.bfloat16
    g = {k: np.asarray(v, f32) for k, v in inputs.items()}

    Wo1m = g["Wo1"] - np.eye(D, dtype=f32)
    bo_msg = g["bo"] + g["bv"] @ g["Wo"]          # bv folded through Wo
    bo_u = bo_msg + g["bo1"]                       # msg bias + ret bias
    bg12 = g["bg1"] + g["bg2"]
    bg3h = 0.5 * g["bg3"]

    wmap = {
        "Wq": g["Wq"], "Wk": g["Wk"], "Wv": g["Wv"], "Wo": g["Wo"],
        "Wo1m": Wo1m, "Wg1": g["Wg1"], "Wg2": g["Wg2"], "Wg3": g["Wg3"],
    }
    bmap = {
        "bq": g["bq"], "bk": g["bk"], "bo_msg": bo_msg,
        "bo_u": bo_u, "bg12": bg12, "bg3h": bg3h,
    }
    wcast = {n: np.ascontiguousarray(w.astype(bf16)) for n, w in wmap.items()}
    return g, wcast, bmap


def _prep_inputs(inputs):
    g, wcast, bmap = _prep_host(inputs)
    modes = {n: _bias_mode(v) for n, v in bmap.items()}
    base = dict(wcast)
    for n, v in bmap.items():
        if modes[n][0] == "ap":
            base[n] = np.ascontiguousarray(v.reshape(D, 1).astype(np.float32))
    x = np.ascontiguousarray(g["x"])
    in_maps = []
    for c in range(NCORES):
        m = dict(base)
        m["x"] = np.ascontiguousarray(x[c * BPC:(c + 1) * BPC])
        in_maps.append(m)
    return in_maps, modes


def kernel(**inputs):
    from concourse.bass_utils import run_bass_kernel_spmd

    in_maps, modes = _prep_inputs(inputs)
    key = tuple(sorted((n, k[0], k[1]) for n, k in modes.items()))
    if _CACHE.get("key") != key:
        _CACHE["nc"] = _build_nc(modes)
        _CACHE["key"] = key
    nc = _CACHE["nc"]

    res = run_bass_kernel_spmd(nc, in_maps, list(range(NCORES)))
    out = np.concatenate([r["out"] for r in res.results], axis=0)
    return out.astype(np.float32)
